# revision 18
# baseline (speedup 1.0000x reference)
"""Trainium2 Bass kernel for nn_CSCFCLayer: out = relu(x @ W + b).

Shapes: x [4096, 4096] f32, W [4096, 4096] f32, b [4096] f32 -> out [4096, 4096] f32.

Sharding: 2D over 8 cores -- batch split 4 ways x units split 2 ways. Each core
computes a [1024, 2048] slice of the output.

v2 design (measured on HW; 302 us baseline -> 218 us):
  - The PE streams 1 moving col/cycle for both f32r and bf16 (a [128x128x512]
    matmul paces at ~213 ns @2.4 GHz), so the PE floor is 1024 MM x 213 ns
    ~= 218 us/core.  bf16 halves HBM traffic (8 MiB xT + 16 MiB W + 8 MiB
    f32 out = 32 MiB/core) and halves LDWEIGHTS time via fast-weight-load;
    bf16 rounding keeps norm rel err ~2.3e-3, well under the 2e-2 gate.
    (fp8 DoubleRow would be 2x on the PE but its ~4e-2 quantization error
    fails the gate, and any error-compensated split needs >=3 half-rate
    passes = slower than one bf16 pass.)
  - All DMAs are fully contiguous: xT, W, and out are pre-packed on the host
    into the exact block order the kernel consumes ([P, KTW, CPG*P] W
    blocks, [P, KTC, BS] xT chunks, [P, 512] out blocks), so every
    descriptor is a multi-KiB per-partition linear run.
  - DMA streams ride separate queues so one stream's semaphore waits never
    block another's issue: xT preload on the SP HWDGE ring, W streaming on
    the Activation HWDGE ring, out writeback on gpsimd SWDGE.
  - MSPLIT/CPG=4: each m-half is its own psum group of 4 banks (2 groups in
    flight), so the moving operand AP stays fixed for 4 consecutive matmuls
    (changing the moving AP costs extra ns/MM).  W is streamed twice (once
    per m-half) but stays hidden under compute.
  - Output is produced transposed (units on partitions) so bias is a
    per-partition scalar: one DVE tensor_scalar (psum + bias, max 0) fuses
    bias+relu while draining PSUM.

Per core: 16 unit-chunks x 2 m-halves x 32 k-tiles = 1024 matmuls.
"""

import os

import numpy as np

import concourse.bass as bass
import concourse.tile as tile
from concourse import bacc, mybir
from concourse.bass_utils import run_bass_kernel_spmd

N_CORES = 8
P_SHARD = 4  # batch split
Q_SHARD = 2  # units split
B = 4096
K = 4096
N = 4096
BS = B // P_SHARD  # 1024 batch rows per core
NS = N // Q_SHARD  # 2048 units per core
P = 128
KT = K // P  # 32 k-tiles
NCHUNK = NS // P  # 16 unit-chunks of 128 per core
MH = BS // 512  # 2 moving halves of the batch

MM_DT = {
    "f32r": mybir.dt.float32r,
    "f32": mybir.dt.float32,
    "bf16": mybir.dt.bfloat16,
}[os.environ.get("CSCFC_MM_DT", "bf16")]

CPG = int(os.environ.get("CSCFC_CPG", "4"))  # unit-chunks per psum group
# MSPLIT: each m-half is its own psum group (moving AP fixed for CPG MMs at
# the cost of streaming W twice; W DMA stays hidden under compute)
MSPLIT = os.environ.get("CSCFC_MSPLIT", "1") == "1"
KTW = min(int(os.environ.get("CSCFC_KTW", "8")), KT)  # k-tiles per W block
KB = KT // KTW  # W blocks per group
NG = NCHUNK // CPG  # psum groups
NBLK = NG * KB  # W blocks per core
NPRE = min(int(os.environ.get("CSCFC_NPRE", "8")), KT)  # xT preload chunks
KTC = KT // NPRE  # k-tiles per xT chunk
WBUFS = int(os.environ.get("CSCFC_WBUFS", "4"))
OBUFS = int(os.environ.get("CSCFC_OBUFS", "8"))
XBUFS = int(os.environ.get("CSCFC_XBUFS", "2"))

# DMA issuing engine per stream: SP HWDGE ring ("sync"), Activation HWDGE
# ring ("act"), or gpsimd SWDGE ("gps").
WENG = os.environ.get("CSCFC_WENG", "act")
XENG = os.environ.get("CSCFC_XENG", "sync")
OENG = os.environ.get("CSCFC_OENG", "gps")


def _dma_eng(nc, which):
    return {"sync": nc.sync, "act": nc.scalar, "gps": nc.gpsimd}[which]


def _emit_loads(nc, xpool, bpool, xt, bt):
    # Resident x.T: [128 (k-partition), 32 (k-tile), 1024 (batch)], loaded in
    # NPRE contiguous chunks so matmuls can start on the first chunk.
    xt_sb = xpool.tile([P, KT, BS], MM_DT, tag="xt_sb", name="xt_sb")
    xeng = _dma_eng(nc, XENG)
    for c in range(NPRE):
        xeng.dma_start(xt_sb[:, c * KTC : (c + 1) * KTC, :], xt.ap()[c])

    bias_sb = bpool.tile([P, NCHUNK], mybir.dt.float32, tag="bias_sb", name="bias_sb")
    nc.sync.dma_start(bias_sb[:], bt.ap()[:, :])
    return xt_sb, bias_sb


def _emit(nc, wpool, opool, psum_pool, xt_sb, bias_sb, w, out):
    # 8 groups of (CPG unit-chunks x MH m-halves) PSUM banks.  j innermost so
    # the moving operand (xt slice) stays fixed for CPG consecutive matmuls.
    # Epilogue: DVE tensor_scalar (psum + per-partition bias, then max 0)
    # drains each bank into half of a [P, 1024] out tile; one 512 KiB
    # contiguous DMA per unit-chunk.
    weng = _dma_eng(nc, WENG)
    oeng = _dma_eng(nc, OENG)
    mh_groups = [[mh] for mh in range(MH)] if MSPLIT else [list(range(MH))]
    for g in range(NG):
        for mhs in mh_groups:
            psums = {
                (mh, j): psum_pool.tile(
                    [P, 512], mybir.dt.float32, tag="ps", name=f"ps_{g}_{mh}_{j}"
                )
                for mh in mhs
                for j in range(CPG)
            }
            for kb in range(KB):
                wt = wpool.tile(
                    [P, KTW, CPG * P], MM_DT, tag="wt", name=f"wt_{g}_{mhs[0]}_{kb}"
                )
                weng.dma_start(wt[:], w.ap()[g * KB + kb])
                for ki in range(KTW):
                    k = kb * KTW + ki
                    for mh in mhs:
                        for j in range(CPG):
                            nc.tensor.matmul(
                                psums[(mh, j)][:],
                                wt[:, ki, j * P : (j + 1) * P],
                                xt_sb[:, k, mh * 512 : (mh + 1) * 512],
                                start=(k == 0),
                                stop=(k == KT - 1),
                            )
            for j in range(CPG):
                ch = g * CPG + j
                if MSPLIT:
                    # one drain+DMA per (chunk, m-half): [P, 512] out blocks
                    for mh in mhs:
                        ot = opool.tile(
                            [P, 512], mybir.dt.float32, tag="ot", name=f"ot_{ch}_{mh}"
                        )
                        nc.vector.tensor_scalar(
                            ot[:],
                            psums[(mh, j)][:],
                            bias_sb[:, ch : ch + 1],
                            0.0,
                            mybir.AluOpType.add,
                            mybir.AluOpType.max,
                        )
                        oeng.dma_start(
                            out.ap()[ch, :, mh * 512 : (mh + 1) * 512], ot[:]
                        )
                else:
                    ot = opool.tile(
                        [P, MH * 512], mybir.dt.float32, tag="ot", name=f"ot_{ch}"
                    )
                    for mh in mhs:
                        nc.vector.tensor_scalar(
                            ot[:, mh * 512 : (mh + 1) * 512],
                            psums[(mh, j)][:],
                            bias_sb[:, ch : ch + 1],
                            0.0,
                            mybir.AluOpType.add,
                            mybir.AluOpType.max,
                        )
                    oeng.dma_start(out.ap()[ch], ot[:])


def build_nc(reps=1, full_reps=1):
    nc = bacc.Bacc("TRN2", target_bir_lowering=False, debug=False)
    xt = nc.dram_tensor("xt", (NPRE, P, KTC * BS), MM_DT, kind="ExternalInput")
    w = nc.dram_tensor("w", (NBLK, P, KTW * CPG * P), MM_DT, kind="ExternalInput")
    bt = nc.dram_tensor("bt", (P, NCHUNK), mybir.dt.float32, kind="ExternalInput")
    out = nc.dram_tensor(
        "out", (NCHUNK, P, MH * 512), mybir.dt.float32, kind="ExternalOutput"
    )
    with tile.TileContext(nc) as tc:
        nc2 = tc.nc
        with (
            tc.tile_pool(name="xpool", bufs=XBUFS) as xpool,
            tc.tile_pool(name="wpool", bufs=WBUFS) as wpool,
            tc.tile_pool(name="bpool", bufs=2) as bpool,
            tc.tile_pool(name="opool", bufs=OBUFS) as opool,
            tc.tile_pool(name="psum", bufs=8, space="PSUM") as psum_pool,
        ):
            if full_reps > 1:
                # timing variant: repeat the ENTIRE kernel (incl. loads).
                # Two reps per For_i body so the xpool (bufs=2) rotation gives
                # ping-pong xt buffers: rep i+1's xt load overlaps rep i's
                # compute, as back-to-back kernel invocations would.
                def body():
                    xt_sb, bias_sb = _emit_loads(nc2, xpool, bpool, xt, bt)
                    _emit(nc2, wpool, opool, psum_pool, xt_sb, bias_sb, w, out)

                unroll = int(os.environ.get("CSCFC_UNROLL", "4"))
                groups_, rem = divmod(full_reps, unroll)
                if groups_ > 0:
                    with tc.For_i(0, groups_, 1):
                        for _ in range(unroll):
                            body()
                for _ in range(rem):
                    body()
            else:
                xt_sb, bias_sb = _emit_loads(nc2, xpool, bpool, xt, bt)
                if reps > 1:
                    # steady-state: resident loads outside the loop
                    with tc.For_i(0, reps, 1):
                        _emit(nc2, wpool, opool, psum_pool, xt_sb, bias_sb, w, out)
                else:
                    _emit(nc2, wpool, opool, psum_pool, xt_sb, bias_sb, w, out)
    nc.compile()
    return nc


_CACHED_NC = None


def _get_nc():
    global _CACHED_NC
    if _CACHED_NC is None:
        _CACHED_NC = build_nc()
    return _CACHED_NC


def make_in_maps(x, w, bias):
    x = np.asarray(x)
    w = np.asarray(w)
    bias = np.asarray(bias)
    np_dt = mybir.dt.np(MM_DT)
    xT = x.T.astype(np_dt)  # [K, B]
    wc = w.astype(np_dt)
    bias = bias.astype(np.float32, copy=False)
    in_maps = []
    for c in range(N_CORES):
        pi, qi = divmod(c, Q_SHARD)
        # xT chunks: [NPRE, P, KTC*BS], chunk c = k-tiles [c*KTC, (c+1)*KTC)
        xc = np.ascontiguousarray(
            xT[:, pi * BS : (pi + 1) * BS]
            .reshape(NPRE, KTC, P, BS)
            .transpose(0, 2, 1, 3)
            .reshape(NPRE, P, KTC * BS)
        )
        # W blocks: [NBLK, P, KTW*CPG*P], block (g, kb) at index g*KB+kb is
        # w[kb*KTW*P:(kb+1)*KTW*P, g*CPG*P:(g+1)*CPG*P] as [P, KTW, CPG*P]
        wcc = (
            wc[:, qi * NS : (qi + 1) * NS]
            .reshape(KB, KTW, P, NG, CPG * P)
            .transpose(3, 0, 2, 1, 4)  # [NG, KB, P, KTW, CPG*P]
            .reshape(NBLK, P, KTW * CPG * P)
        )
        in_maps.append(
            {
                "xt": xc,
                "w": np.ascontiguousarray(wcc),
                "bt": np.ascontiguousarray(
                    bias[qi * NS : (qi + 1) * NS].reshape(NCHUNK, P).T
                ),
            }
        )
    return in_maps


def gather_out(results):
    out = np.empty((B, N), dtype=np.float32)
    for c in range(N_CORES):
        pi, qi = divmod(c, Q_SHARD)
        # r: [NCHUNK, P, MH*512] -- r[ch, u, m] = out[pi*BS + m, qi*NS + ch*P + u]
        r = results[c]["out"]
        out[pi * BS : (pi + 1) * BS, qi * NS : (qi + 1) * NS] = (
            r.transpose(2, 0, 1).reshape(BS, NS)
        )
    return out


def _run(nc, x, w, bias, **spmd_kwargs):
    in_maps = make_in_maps(x, w, bias)
    res = run_bass_kernel_spmd(nc, in_maps, list(range(N_CORES)), **spmd_kwargs)
    return gather_out(res.results), res


def kernel(x, kernel, bias):
    try:
        out, _ = _run(_get_nc(), x, kernel, bias)
    except Exception:
        # transient device errors (e.g. NRT_EXEC_UNIT_UNRECOVERABLE) recover
        # on re-execution
        out, _ = _run(_get_nc(), x, kernel, bias)
    return out


# revision 19
# speedup vs baseline: 1.0472x; 1.0472x over previous
"""Trainium2 Bass kernel for nn_CSCFCLayer: out = relu(x @ W + b).

Shapes: x [4096, 4096] f32, W [4096, 4096] f32, b [4096] f32 -> out [4096, 4096] f32.

Sharding: 2D over 8 cores -- batch split 4 ways x units split 2 ways. Each core
computes a [1024, 2048] slice of the output.

v2 design (measured on HW; 302 us baseline -> 218 us):
  - The PE streams 1 moving col/cycle for both f32r and bf16 (a [128x128x512]
    matmul paces at ~213 ns @2.4 GHz), so the PE floor is 1024 MM x 213 ns
    ~= 218 us/core.  bf16 halves HBM traffic (8 MiB xT + 16 MiB W + 8 MiB
    f32 out = 32 MiB/core) and halves LDWEIGHTS time via fast-weight-load;
    bf16 rounding keeps norm rel err ~2.3e-3, well under the 2e-2 gate.
    (fp8 DoubleRow would be 2x on the PE but its ~4e-2 quantization error
    fails the gate, and any error-compensated split needs >=3 half-rate
    passes = slower than one bf16 pass.)
  - All DMAs are fully contiguous: xT, W, and out are pre-packed on the host
    into the exact block order the kernel consumes ([P, KTW, CPG*P] W
    blocks, [P, KTC, BS] xT chunks, [P, 512] out blocks), so every
    descriptor is a multi-KiB per-partition linear run.
  - DMA streams ride separate queues so one stream's semaphore waits never
    block another's issue: xT preload on the SP HWDGE ring, W streaming on
    the Activation HWDGE ring, out writeback on gpsimd SWDGE.
  - MSPLIT/CPG=4: each m-half is its own psum group of 4 banks (2 groups in
    flight), so the moving operand AP stays fixed for 4 consecutive matmuls
    (changing the moving AP costs extra ns/MM).  W is streamed twice (once
    per m-half) but stays hidden under compute.
  - Output is produced transposed (units on partitions) so bias is a
    per-partition scalar: one DVE tensor_scalar (psum + bias, max 0) fuses
    bias+relu while draining PSUM.

Per core: 16 unit-chunks x 2 m-halves x 32 k-tiles = 1024 matmuls.
"""

import os

import numpy as np

import concourse.bass as bass
import concourse.tile as tile
from concourse import bacc, mybir
from concourse.bass_utils import run_bass_kernel_spmd

N_CORES = 8
P_SHARD = 4  # batch split
Q_SHARD = 2  # units split
B = 4096
K = 4096
N = 4096
BS = B // P_SHARD  # 1024 batch rows per core
NS = N // Q_SHARD  # 2048 units per core
P = 128
KT = K // P  # 32 k-tiles
NCHUNK = NS // P  # 16 unit-chunks of 128 per core
MH = BS // 512  # 2 moving halves of the batch

MM_DT = {
    "f32r": mybir.dt.float32r,
    "f32": mybir.dt.float32,
    "bf16": mybir.dt.bfloat16,
}[os.environ.get("CSCFC_MM_DT", "bf16")]

CPG = int(os.environ.get("CSCFC_CPG", "4"))  # unit-chunks per psum group
# MSPLIT: each m-half is its own psum group (moving AP fixed for CPG MMs at
# the cost of streaming W twice; W DMA stays hidden under compute)
MSPLIT = os.environ.get("CSCFC_MSPLIT", "1") == "1"
KTW = min(int(os.environ.get("CSCFC_KTW", "8")), KT)  # k-tiles per W block
KB = KT // KTW  # W blocks per group
NG = NCHUNK // CPG  # psum groups
NBLK = NG * KB  # W blocks per core
NPRE = min(int(os.environ.get("CSCFC_NPRE", "8")), KT)  # xT preload chunks
KTC = KT // NPRE  # k-tiles per xT chunk
WBUFS = int(os.environ.get("CSCFC_WBUFS", "4"))
OBUFS = int(os.environ.get("CSCFC_OBUFS", "8"))
XBUFS = int(os.environ.get("CSCFC_XBUFS", "2"))

# DMA issuing engine per stream: SP HWDGE ring ("sync"), Activation HWDGE
# ring ("act"), or gpsimd SWDGE ("gps").
WENG = os.environ.get("CSCFC_WENG", "act")
XENG = os.environ.get("CSCFC_XENG", "sync")
OENG = os.environ.get("CSCFC_OENG", "gps")


def _dma_eng(nc, which):
    return {"sync": nc.sync, "act": nc.scalar, "gps": nc.gpsimd}[which]


def _emit_loads(nc, xpool, bpool, xt, bt):
    # Resident x.T: [128 (k-partition), 32 (k-tile), 1024 (batch)], loaded in
    # NPRE contiguous chunks so matmuls can start on the first chunk.
    xt_sb = xpool.tile([P, KT, BS], MM_DT, tag="xt_sb", name="xt_sb")
    xeng = _dma_eng(nc, XENG)
    for c in range(NPRE):
        xeng.dma_start(xt_sb[:, c * KTC : (c + 1) * KTC, :], xt.ap()[c])

    bias_sb = bpool.tile([P, NCHUNK], mybir.dt.float32, tag="bias_sb", name="bias_sb")
    nc.sync.dma_start(bias_sb[:], bt.ap()[:, :])
    return xt_sb, bias_sb


def _emit(nc, wpool, opool, psum_pool, xt_sb, bias_sb, w, out):
    # 8 groups of (CPG unit-chunks x MH m-halves) PSUM banks.  j innermost so
    # the moving operand (xt slice) stays fixed for CPG consecutive matmuls.
    # Epilogue: DVE tensor_scalar (psum + per-partition bias, then max 0)
    # drains each bank into half of a [P, 1024] out tile; one 512 KiB
    # contiguous DMA per unit-chunk.
    weng = _dma_eng(nc, WENG)
    oeng = _dma_eng(nc, OENG)
    mh_groups = [[mh] for mh in range(MH)] if MSPLIT else [list(range(MH))]
    for g in range(NG):
        for mhs in mh_groups:
            psums = {
                (mh, j): psum_pool.tile(
                    [P, 512], mybir.dt.float32, tag="ps", name=f"ps_{g}_{mh}_{j}"
                )
                for mh in mhs
                for j in range(CPG)
            }
            for kb in range(KB):
                wt = wpool.tile(
                    [P, KTW, CPG * P], MM_DT, tag="wt", name=f"wt_{g}_{mhs[0]}_{kb}"
                )
                weng.dma_start(wt[:], w.ap()[g * KB + kb])
                for ki in range(KTW):
                    k = kb * KTW + ki
                    for mh in mhs:
                        for j in range(CPG):
                            nc.tensor.matmul(
                                psums[(mh, j)][:],
                                wt[:, ki, j * P : (j + 1) * P],
                                xt_sb[:, k, mh * 512 : (mh + 1) * 512],
                                start=(k == 0),
                                stop=(k == KT - 1),
                            )
            for j in range(CPG):
                ch = g * CPG + j
                if MSPLIT:
                    # one drain+DMA per (chunk, m-half): [P, 512] out blocks
                    for mh in mhs:
                        ot = opool.tile(
                            [P, 512], mybir.dt.float32, tag="ot", name=f"ot_{ch}_{mh}"
                        )
                        nc.vector.tensor_scalar(
                            ot[:],
                            psums[(mh, j)][:],
                            bias_sb[:, ch : ch + 1],
                            0.0,
                            mybir.AluOpType.add,
                            mybir.AluOpType.max,
                        )
                        oeng.dma_start(
                            out.ap()[ch, :, mh * 512 : (mh + 1) * 512], ot[:]
                        )
                else:
                    ot = opool.tile(
                        [P, MH * 512], mybir.dt.float32, tag="ot", name=f"ot_{ch}"
                    )
                    for mh in mhs:
                        nc.vector.tensor_scalar(
                            ot[:, mh * 512 : (mh + 1) * 512],
                            psums[(mh, j)][:],
                            bias_sb[:, ch : ch + 1],
                            0.0,
                            mybir.AluOpType.add,
                            mybir.AluOpType.max,
                        )
                    oeng.dma_start(out.ap()[ch], ot[:])


def build_nc(reps=1, full_reps=1):
    nc = bacc.Bacc("TRN2", target_bir_lowering=False, debug=False)
    xt = nc.dram_tensor("xt", (NPRE, P, KTC * BS), MM_DT, kind="ExternalInput")
    w = nc.dram_tensor("w", (NBLK, P, KTW * CPG * P), MM_DT, kind="ExternalInput")
    bt = nc.dram_tensor("bt", (P, NCHUNK), mybir.dt.float32, kind="ExternalInput")
    out = nc.dram_tensor(
        "out", (NCHUNK, P, MH * 512), mybir.dt.float32, kind="ExternalOutput"
    )
    with tile.TileContext(nc) as tc:
        nc2 = tc.nc
        with (
            tc.tile_pool(name="xpool", bufs=XBUFS) as xpool,
            tc.tile_pool(name="wpool", bufs=WBUFS) as wpool,
            tc.tile_pool(name="bpool", bufs=2) as bpool,
            tc.tile_pool(name="opool", bufs=OBUFS) as opool,
            tc.tile_pool(name="psum", bufs=8, space="PSUM") as psum_pool,
        ):
            if full_reps > 1:
                # timing variant: repeat the ENTIRE kernel (incl. loads).
                # Two reps per For_i body so the xpool (bufs=2) rotation gives
                # ping-pong xt buffers: rep i+1's xt load overlaps rep i's
                # compute, as back-to-back kernel invocations would.
                def body():
                    xt_sb, bias_sb = _emit_loads(nc2, xpool, bpool, xt, bt)
                    _emit(nc2, wpool, opool, psum_pool, xt_sb, bias_sb, w, out)

                unroll = int(os.environ.get("CSCFC_UNROLL", "2"))
                groups_, rem = divmod(full_reps, unroll)
                if groups_ > 0:
                    with tc.For_i(0, groups_, 1):
                        for _ in range(unroll):
                            body()
                for _ in range(rem):
                    body()
            else:
                xt_sb, bias_sb = _emit_loads(nc2, xpool, bpool, xt, bt)
                if reps > 1:
                    # steady-state: resident loads outside the loop
                    with tc.For_i(0, reps, 1):
                        _emit(nc2, wpool, opool, psum_pool, xt_sb, bias_sb, w, out)
                else:
                    _emit(nc2, wpool, opool, psum_pool, xt_sb, bias_sb, w, out)
    nc.compile()
    return nc


_CACHED_NC = None


def _get_nc():
    global _CACHED_NC
    if _CACHED_NC is None:
        _CACHED_NC = build_nc()
    return _CACHED_NC


def make_in_maps(x, w, bias):
    x = np.asarray(x)
    w = np.asarray(w)
    bias = np.asarray(bias)
    np_dt = mybir.dt.np(MM_DT)
    xT = x.T.astype(np_dt)  # [K, B]
    wc = w.astype(np_dt)
    bias = bias.astype(np.float32, copy=False)
    in_maps = []
    for c in range(N_CORES):
        pi, qi = divmod(c, Q_SHARD)
        # xT chunks: [NPRE, P, KTC*BS], chunk c = k-tiles [c*KTC, (c+1)*KTC)
        xc = np.ascontiguousarray(
            xT[:, pi * BS : (pi + 1) * BS]
            .reshape(NPRE, KTC, P, BS)
            .transpose(0, 2, 1, 3)
            .reshape(NPRE, P, KTC * BS)
        )
        # W blocks: [NBLK, P, KTW*CPG*P], block (g, kb) at index g*KB+kb is
        # w[kb*KTW*P:(kb+1)*KTW*P, g*CPG*P:(g+1)*CPG*P] as [P, KTW, CPG*P]
        wcc = (
            wc[:, qi * NS : (qi + 1) * NS]
            .reshape(KB, KTW, P, NG, CPG * P)
            .transpose(3, 0, 2, 1, 4)  # [NG, KB, P, KTW, CPG*P]
            .reshape(NBLK, P, KTW * CPG * P)
        )
        in_maps.append(
            {
                "xt": xc,
                "w": np.ascontiguousarray(wcc),
                "bt": np.ascontiguousarray(
                    bias[qi * NS : (qi + 1) * NS].reshape(NCHUNK, P).T
                ),
            }
        )
    return in_maps


def gather_out(results):
    out = np.empty((B, N), dtype=np.float32)
    for c in range(N_CORES):
        pi, qi = divmod(c, Q_SHARD)
        # r: [NCHUNK, P, MH*512] -- r[ch, u, m] = out[pi*BS + m, qi*NS + ch*P + u]
        r = results[c]["out"]
        out[pi * BS : (pi + 1) * BS, qi * NS : (qi + 1) * NS] = (
            r.transpose(2, 0, 1).reshape(BS, NS)
        )
    return out


def _run(nc, x, w, bias, **spmd_kwargs):
    in_maps = make_in_maps(x, w, bias)
    res = run_bass_kernel_spmd(nc, in_maps, list(range(N_CORES)), **spmd_kwargs)
    return gather_out(res.results), res


def kernel(x, kernel, bias):
    try:
        out, _ = _run(_get_nc(), x, kernel, bias)
    except Exception:
        # transient device errors (e.g. NRT_EXEC_UNIT_UNRECOVERABLE) recover
        # on re-execution
        out, _ = _run(_get_nc(), x, kernel, bias)
    return out
